# revision 9
# baseline (speedup 1.0000x reference)
"""Trainium2 Bass kernel for an 8-expert MoE FFN layer (nn_MoELayer).

Reference computation (per expert e over its contiguous 1024-token chunk):
    h = gelu(x_e @ w1[e] + b1[e]);  y_e = h @ w2[e] + b2[e]

Sharding: expert parallelism — core e holds expert e's weights and its token
chunk (the gate yields equal contiguous chunks, so no all-to-all is needed).
Each core runs the same SPMD program on its own data.

Per-core kernel (T=1024 tokens, D=1024, F=4096), all matmuls in fp16 with
fp32 PSUM accumulation (~216 ns per 512-wide matmul incl. hidden weight load —
the PE's floor). 1024 matmuls -> ~220.7 us of PE stream; everything else is
head/tail overlap engineering:
  - warmup matmuls on (uninitialized) scratch start right after the engine
    preamble, so HAM un-throttles the PE clock before real data lands
  - critical input DMAs (w1[0..2], x chunk-0 quarters) issue in parallel from
    four different engine queues to cut descriptor-issue serialization
  - phase 1 runs chunk-major (all 32 f-tiles on chunk 0, then chunk 1) with
    all of w1 SBUF-resident, so the head only waits for w1[0] + x chunk-0
    quarters (~0.5 MB) instead of both chunks (~2.25 MB)
  - phase 2: per dm-tile 2-bank PSUM accumulate; the last dm-tile runs
    chunk-major on 1-bank psum tiles so its first chunk's epilogue overlaps
    the second chunk's matmuls, and the final flush is split across the
    vector+scalar engines with DMA issue on sync+gpsimd in parallel.
"""

import os

import numpy as np

# The kernel executes through the axon PJRT backend; a CPU pin (e.g. set for
# a jax reference run) would break NEFF dispatch in this process.
if os.environ.get("JAX_PLATFORMS") == "cpu":
    del os.environ["JAX_PLATFORMS"]

E = 8          # experts == cores
B, S = 2, 4096
D = 1024       # d_model
F = 4096       # d_ff
T = (B * S) // E  # tokens per expert chunk = 1024
P = 128
DO = D // P    # 8  k-tiles of d_model
FT = F // P    # 32 f-tiles of d_ff
DMO = D // P   # 8  output dm-tiles
FT2 = FT // 2  # half-slab of w2 f-tiles
NCHUNK = T // 512  # 2 moving-operand chunks (PSUM bank caps matmul N at 512)
N_WARM_BIG = 8     # N=512 warmups: ~3.4us cold -> trips the HAM un-throttle
N_WARM_SMALL = 6   # N=128 warmups: fine-grained handoff to the first real mm

_cached = None


def _build():
    import concourse.mybir as mybir
    import concourse.tile as tile
    from concourse import bacc
    from concourse.tile_rust import add_dep_helper

    f32 = mybir.dt.float32
    f16 = mybir.dt.float16

    nc = bacc.Bacc("TRN2", target_bir_lowering=False, debug=False, num_devices=E)

    xT_d = nc.dram_tensor("xT", [NCHUNK, P, DO, 512], f16, kind="ExternalInput")
    w1_d = nc.dram_tensor("w1r", [FT, P, DO, P], f16, kind="ExternalInput")
    bc_d = nc.dram_tensor("bc", [P, FT + DMO], f32, kind="ExternalInput")
    w2_d = nc.dram_tensor("w2r", [DMO, 2, P, FT2, P], f16, kind="ExternalInput")
    yT_d = nc.dram_tensor("yT", [DMO, P, T], f32, kind="ExternalOutput")

    gelu = mybir.ActivationFunctionType.Gelu_apprx_tanh
    ident = mybir.ActivationFunctionType.Identity

    with tile.TileContext(nc) as tc:
        with (
            tc.tile_pool(name="xpool", bufs=1) as xpool,
            tc.tile_pool(name="hpool", bufs=1) as hpool,
            tc.tile_pool(name="wpool", bufs=2) as wpool,
            tc.tile_pool(name="cpool", bufs=1) as cpool,
            tc.tile_pool(name="ypool", bufs=2) as ypool,
            tc.tile_pool(name="psum_h", bufs=4, space="PSUM") as psum_h,
            tc.tile_pool(name="psum_y", bufs=2, space="PSUM") as psum_y,
        ):
            # Scratch for PE warmup, zeroed on the vector engine (idle at the
            # head, and not a DMA-capable engine) so the warmup matmuls can
            # start right after the preamble without gating any input DMA.
            scratch = cpool.tile([P, 512], f16)
            nc.vector.memset(scratch[:], 0.0)

            # ---- critical input DMAs, issued in parallel across engines.
            # gpsimd: w1[0..2] + x chunk 1; vector/scalar: x chunk-0 quarters.
            # sync: the long w1 stream. One engine issuing everything would
            # serialize ~600ns per descriptor right on the critical path.
            w1_tiles = []
            for ft in range(FT):
                w1_tiles.append(
                    wpool.tile([P, DO, P], f16, tag="w1", bufs=FT, name="w1_sb")
                )
            xT_sb = xpool.tile([P, NCHUNK, DO, 512], f16)

            # Strict need-order across the two fastest-issuing engines: the
            # first matmul group consumes (w1[0] do-half 0, x q0, q1, w1[0]
            # do-half 1, q2, q3), then w1[1], w1[2], ... Everything beyond a
            # few tiles of lookahead is gated behind phase-1 progress (below)
            # so its packets can't crowd the critical head window.
            nc.scalar.dma_start(w1_tiles[0][:, 0:4, :], w1_d.ap()[0][:, 0:4, :])
            nc.sync.dma_start(xT_sb[:, 0, 0:2, :], xT_d.ap()[0][:, 0:2, :])
            nc.scalar.dma_start(xT_sb[:, 0, 2:4, :], xT_d.ap()[0][:, 2:4, :])
            nc.sync.dma_start(w1_tiles[0][:, 4:8, :], w1_d.ap()[0][:, 4:8, :])
            nc.scalar.dma_start(xT_sb[:, 0, 4:6, :], xT_d.ap()[0][:, 4:6, :])
            nc.sync.dma_start(xT_sb[:, 0, 6:8, :], xT_d.ap()[0][:, 6:8, :])
            nc.scalar.dma_start(w1_tiles[1][:], w1_d.ap()[1])
            nc.sync.dma_start(w1_tiles[2][:], w1_d.ap()[2])
            bc_sb = cpool.tile([P, FT + DMO], f32)
            nc.scalar.dma_start(bc_sb[:], bc_d.ap())
            w1_dmas = {}
            for ft in range(3, FT):
                w1_dmas[ft] = nc.sync.dma_start(w1_tiles[ft][:], w1_d.ap()[ft])
            b1_sb = bc_sb[:, :FT]
            b2_sb = bc_sb[:, FT:]
            # x chunk 1 is only needed by the second phase-1 pass (~110us in);
            # gpsimd issues it once phase 1 is underway (gated below).
            xc1_dmas = [
                nc.gpsimd.dma_start(xT_sb[:, 1, 0:4, :], xT_d.ap()[1][:, 0:4, :]),
                nc.gpsimd.dma_start(xT_sb[:, 1, 4:8, :], xT_d.ap()[1][:, 4:8, :]),
            ]

            # ---- PE warmup on scratch while the first DMAs are in flight.
            # Keeps the HAM clock-gate at 2.4 GHz by the time real work lands.
            for i in range(N_WARM_BIG):
                pw = psum_h.tile([P, 512], f32, tag="ph", name="pwarm")
                nc.tensor.matmul(
                    pw[:], scratch[:, :P], scratch[:], start=True, stop=True
                )
            for i in range(N_WARM_SMALL):
                pw = psum_h.tile([P, 512], f32, tag="ph", name="pwarm")
                nc.tensor.matmul(
                    pw[:, :P], scratch[:, :P], scratch[:, :P], start=True, stop=True
                )

            h_sb = hpool.tile([P, FT, T], f16)

            # ---- phase 1, chunk-major: h^T = gelu(w1^T x^T + b1).
            # All w1 tiles stay resident, so chunk 0 only needs x chunk 0 and
            # w1[ft] just-in-time; x chunk 1 has ~110us of slack.
            gelu_insts = {}
            for c in range(NCHUNK):
                cs = slice(c * 512, (c + 1) * 512)
                for ft in range(FT):
                    ph = psum_h.tile([P, 512], f32, tag="ph", name="ph")
                    for do in range(DO):
                        nc.tensor.matmul(
                            ph[:],
                            w1_tiles[ft][:, do, :],
                            xT_sb[:, c, do, :],
                            start=(do == 0),
                            stop=(do == DO - 1),
                        )
                    gelu_insts[(ft, c)] = nc.scalar.activation(
                        h_sb[:, ft, cs], ph[:], gelu, bias=b1_sb[:, ft : ft + 1]
                    )

            # Pace the non-critical input DMAs behind phase-1 progress: w1[ft]
            # issues once f-tile ft-5 of chunk 0 is done (5 tiles of
            # lookahead), x chunk 1 once the head has cleared. Without this
            # their packets queue ahead of the just-in-time critical tiles.
            for ft in range(5, FT):
                add_dep_helper(
                    w1_dmas[ft].ins,
                    gelu_insts[(ft - 5, 0)].ins,
                    sync=True,
                    reason="pace w1 stream to phase-1 consumption",
                )
            for dma in xc1_dmas:
                add_dep_helper(
                    dma.ins,
                    gelu_insts[(1, 0)].ins,
                    sync=True,
                    reason="keep x chunk 1 out of the head window",
                )

            # ---- phase 2: y^T[dmo] = w2[:,dmo]^T h^T + b2[dmo]
            FQ = FT // 4
            for dmo in range(DMO):
                w2_q = []
                for qq in range(4):
                    w2_sb = wpool.tile([P, FQ, P], f16, tag="w2", bufs=8, name="w2_sb")
                    dma = nc.gpsimd.dma_start(
                        w2_sb[:],
                        w2_d.ap()[dmo, qq // 2, :, (qq % 2) * FQ : (qq % 2 + 1) * FQ],
                    )
                    if dmo == 0 and qq == 0:
                        # keep the w2 stream out of the head's w1/x DMA window
                        add_dep_helper(
                            dma.ins,
                            gelu_insts[(8, 0)].ins,
                            sync=True,
                            reason="delay w2 prefetch past the kernel head",
                        )
                    w2_q.append(w2_sb)

                if dmo < DMO - 1:
                    py = psum_y.tile([P, T], f32, tag="py", name="py")
                    for fo in range(FT):
                        wt = w2_q[fo // FQ][:, fo % FQ, :]
                        for c in range(NCHUNK):
                            cs = slice(c * 512, (c + 1) * 512)
                            nc.tensor.matmul(
                                py[:, cs],
                                wt,
                                h_sb[:, fo, cs],
                                start=(fo == 0),
                                stop=(fo == FT - 1),
                            )
                    # bias-add + store in 256 chunks so the DMA overlaps the add
                    for cq in range(4):
                        cs = slice(cq * 256, (cq + 1) * 256)
                        y_sb = ypool.tile([P, 256], f32, tag="y", bufs=4, name="y_sb")
                        nc.vector.tensor_scalar_add(
                            y_sb[:], py[:, cs], b2_sb[:, dmo : dmo + 1]
                        )
                        nc.sync.dma_start(yT_d.ap()[dmo, :, cs], y_sb[:])
                else:
                    # last dm-tile: chunk-major on 1-bank psum tiles (from the
                    # phase-1 pool, long dead) so chunk 0's epilogue overlaps
                    # chunk 1's matmuls. The final adds run on vector+scalar
                    # in parallel; all stores issue from sync (gpsimd wakes up
                    # too slowly for the tail-critical store).
                    def last_mm_chunk(c):
                        py_c = psum_h.tile([P, 512], f32, tag="ph", name="py_c")
                        mms = []
                        for fo in range(FT):
                            wt = w2_q[fo // FQ][:, fo % FQ, :]
                            mms.append(
                                nc.tensor.matmul(
                                    py_c[:],
                                    wt,
                                    h_sb[:, fo, c * 512 : (c + 1) * 512],
                                    start=(fo == 0),
                                    stop=(fo == FT - 1),
                                )
                            )
                        return py_c, mms

                    def last_flush(py_c, c):
                        for half in range(2):
                            hs = slice(half * 256, (half + 1) * 256)
                            ds = slice(c * 512 + half * 256, c * 512 + half * 256 + 256)
                            y_sb = ypool.tile(
                                [P, 256], f32, tag="y", bufs=4, name="y_sb"
                            )
                            if half == 0:
                                nc.vector.tensor_scalar_add(
                                    y_sb[:], py_c[:, hs], b2_sb[:, dmo : dmo + 1]
                                )
                            else:
                                nc.scalar.activation(
                                    y_sb[:], py_c[:, hs], ident,
                                    bias=b2_sb[:, dmo : dmo + 1],
                                )
                            nc.sync.dma_start(yT_d.ap()[dmo, :, ds], y_sb[:])

                    py_c0, _ = last_mm_chunk(0)
                    last_flush(py_c0, 0)
                    py_c1, c1_mms = last_mm_chunk(1)
                    # ring-warm trickles: tiny loads issued mid-way through
                    # chunk 1's matmuls keep the DMA rings awake so the
                    # tail-critical final stores skip the wake-up latency.
                    warm_sb = cpool.tile([P, 16], f16, name="warm_sb")
                    for k, gate in ((0, 12), (1, 22)):
                        tdma = nc.sync.dma_start(
                            warm_sb[:, 8 * k : 8 * k + 8],
                            xT_d.ap()[0][:, 0, 8 * k : 8 * k + 8],
                        )
                        add_dep_helper(
                            tdma.ins,
                            c1_mms[gate].ins,
                            sync=True,
                            reason="ring-warm trickle before final store",
                        )
                    last_flush(py_c1, 1)

    nc.compile()
    return nc


def _get_nc():
    global _cached
    if _cached is None:
        _cached = _build()
    return _cached


def make_in_maps(x, w1, b1, w2, b2):
    x = np.asarray(x, dtype=np.float32)
    w1 = np.asarray(w1, dtype=np.float32)
    b1 = np.asarray(b1, dtype=np.float32)
    w2 = np.asarray(w2, dtype=np.float32)
    b2 = np.asarray(b2, dtype=np.float32)

    tokens = x.reshape(E, T, D)
    in_maps = []
    for e in range(E):
        xT = np.ascontiguousarray(
            tokens[e].reshape(NCHUNK, 512, DO, P).transpose(0, 3, 2, 1)
        ).astype(np.float16)  # [c, p, do, t']
        w1r = np.ascontiguousarray(
            w1[e].reshape(DO, P, FT, P).transpose(2, 1, 0, 3)
        ).astype(np.float16)  # [ft, p, do, j]
        bc = np.ascontiguousarray(
            np.concatenate([b1[e].reshape(FT, P).T, b2[e].reshape(DMO, P).T], axis=1)
        )  # [p, ft..dmo]
        w2r = np.ascontiguousarray(
            w2[e].reshape(2, FT2, P, DMO, P).transpose(3, 0, 2, 1, 4)
        ).astype(np.float16)  # [dmo, half, p, fo, j]
        in_maps.append({"xT": xT, "w1r": w1r, "bc": bc, "w2r": w2r})
    return in_maps


def gather_out(results):
    out = np.empty((E, T, D), dtype=np.float32)
    for e in range(E):
        yT = results[e]["yT"]  # [dmo, p, t]
        out[e] = yT.transpose(2, 0, 1).reshape(T, D)
    return out.reshape(B, S, D)


def kernel(x, w1, b1, w2, b2):
    from concourse.bass_utils import run_bass_kernel_spmd

    nc = _get_nc()
    in_maps = make_in_maps(x, w1, b1, w2, b2)
    res = run_bass_kernel_spmd(nc, in_maps, core_ids=list(range(E)))
    return gather_out(res.results)


# revision 10
# speedup vs baseline: 1.0239x; 1.0239x over previous
"""Trainium2 Bass kernel for an 8-expert MoE FFN layer (nn_MoELayer).

Reference computation (per expert e over its contiguous 1024-token chunk):
    h = gelu(x_e @ w1[e] + b1[e]);  y_e = h @ w2[e] + b2[e]

Sharding: expert parallelism — core e holds expert e's weights and its token
chunk (the gate yields equal contiguous chunks, so no all-to-all is needed).
Each core runs the same SPMD program on its own data.

Per-core kernel (T=1024 tokens, D=1024, F=4096), all matmuls in fp16 with
fp32 PSUM accumulation (~216 ns per 512-wide matmul incl. hidden weight load —
the PE's floor). 1024 matmuls -> ~220.7 us of PE stream; everything else is
head/tail overlap engineering:
  - warmup matmuls on (uninitialized) scratch start right after the engine
    preamble, so HAM un-throttles the PE clock before real data lands
  - critical input DMAs (w1[0..2], x chunk-0 quarters) issue in parallel from
    four different engine queues to cut descriptor-issue serialization
  - phase 1 runs chunk-major (all 32 f-tiles on chunk 0, then chunk 1) with
    all of w1 SBUF-resident, so the head only waits for w1[0] + x chunk-0
    quarters (~0.5 MB) instead of both chunks (~2.25 MB)
  - phase 2: per dm-tile 2-bank PSUM accumulate; the last dm-tile runs
    chunk-major on 1-bank psum tiles so its first chunk's epilogue overlaps
    the second chunk's matmuls, and the final flush is split across the
    vector+scalar engines with DMA issue on sync+gpsimd in parallel.
"""

import os

import numpy as np

# The kernel executes through the axon PJRT backend; a CPU pin (e.g. set for
# a jax reference run) would break NEFF dispatch in this process.
if os.environ.get("JAX_PLATFORMS") == "cpu":
    del os.environ["JAX_PLATFORMS"]

E = 8          # experts == cores
B, S = 2, 4096
D = 1024       # d_model
F = 4096       # d_ff
T = (B * S) // E  # tokens per expert chunk = 1024
P = 128
DO = D // P    # 8  k-tiles of d_model
FT = F // P    # 32 f-tiles of d_ff
DMO = D // P   # 8  output dm-tiles
FT2 = FT // 2  # half-slab of w2 f-tiles
NCHUNK = T // 512  # 2 moving-operand chunks (PSUM bank caps matmul N at 512)
N_WARM_BIG = 8     # N=512 warmups: ~3.4us cold -> trips the HAM un-throttle
N_WARM_SMALL = 6   # N=128 warmups: fine-grained handoff to the first real mm

_cached = None


def _build():
    import concourse.mybir as mybir
    import concourse.tile as tile
    from concourse import bacc
    from concourse.tile_rust import add_dep_helper

    f32 = mybir.dt.float32
    f16 = mybir.dt.float16

    nc = bacc.Bacc("TRN2", target_bir_lowering=False, debug=False, num_devices=E)

    xT_d = nc.dram_tensor("xT", [NCHUNK, P, DO, 512], f16, kind="ExternalInput")
    w1_d = nc.dram_tensor("w1r", [FT, P, DO, P], f16, kind="ExternalInput")
    bc_d = nc.dram_tensor("bc", [P, FT + DMO], f32, kind="ExternalInput")
    w2_d = nc.dram_tensor("w2r", [DMO, 2, P, FT2, P], f16, kind="ExternalInput")
    yT_d = nc.dram_tensor("yT", [DMO, P, T], f32, kind="ExternalOutput")

    gelu = mybir.ActivationFunctionType.Gelu_apprx_tanh
    ident = mybir.ActivationFunctionType.Identity

    with tile.TileContext(nc) as tc:
        with (
            tc.tile_pool(name="xpool", bufs=1) as xpool,
            tc.tile_pool(name="hpool", bufs=1) as hpool,
            tc.tile_pool(name="wpool", bufs=2) as wpool,
            tc.tile_pool(name="cpool", bufs=1) as cpool,
            tc.tile_pool(name="ypool", bufs=2) as ypool,
            tc.tile_pool(name="psum_h", bufs=4, space="PSUM") as psum_h,
            tc.tile_pool(name="psum_y", bufs=2, space="PSUM") as psum_y,
        ):
            # Scratch for PE warmup, zeroed on the vector engine (idle at the
            # head, and not a DMA-capable engine) so the warmup matmuls can
            # start right after the preamble without gating any input DMA.
            scratch = cpool.tile([P, 512], f16)
            nc.vector.memset(scratch[:], 0.0)

            # ---- critical input DMAs, issued in parallel across engines.
            # gpsimd: w1[0..2] + x chunk 1; vector/scalar: x chunk-0 quarters.
            # sync: the long w1 stream. One engine issuing everything would
            # serialize ~600ns per descriptor right on the critical path.
            w1_tiles = []
            for ft in range(FT):
                w1_tiles.append(
                    wpool.tile([P, DO, P], f16, tag="w1", bufs=FT, name="w1_sb")
                )
            xT_sb = xpool.tile([P, NCHUNK, DO, 512], f16)

            # Strict need-order across the two fastest-issuing engines: the
            # first matmul group consumes (w1[0] do-half 0, x q0, q1, w1[0]
            # do-half 1, q2, q3), then w1[1], w1[2], ... Everything beyond a
            # few tiles of lookahead is gated behind phase-1 progress (below)
            # so its packets can't crowd the critical head window.
            nc.scalar.dma_start(w1_tiles[0][:, 0:4, :], w1_d.ap()[0][:, 0:4, :])
            nc.sync.dma_start(xT_sb[:, 0, 0:2, :], xT_d.ap()[0][:, 0:2, :])
            nc.scalar.dma_start(xT_sb[:, 0, 2:4, :], xT_d.ap()[0][:, 2:4, :])
            nc.sync.dma_start(w1_tiles[0][:, 4:8, :], w1_d.ap()[0][:, 4:8, :])
            nc.scalar.dma_start(xT_sb[:, 0, 4:6, :], xT_d.ap()[0][:, 4:6, :])
            nc.sync.dma_start(xT_sb[:, 0, 6:8, :], xT_d.ap()[0][:, 6:8, :])
            nc.scalar.dma_start(w1_tiles[1][:], w1_d.ap()[1])
            nc.sync.dma_start(w1_tiles[2][:], w1_d.ap()[2])
            bc_sb = cpool.tile([P, FT + DMO], f32)
            nc.scalar.dma_start(bc_sb[:], bc_d.ap())
            w1_dmas = {}
            for ft in range(3, FT):
                w1_dmas[ft] = nc.sync.dma_start(w1_tiles[ft][:], w1_d.ap()[ft])
            b1_sb = bc_sb[:, :FT]
            b2_sb = bc_sb[:, FT:]
            # x chunk 1 is only needed by the second phase-1 pass (~110us in);
            # gpsimd issues it once phase 1 is underway (gated below).
            xc1_dmas = [
                nc.gpsimd.dma_start(xT_sb[:, 1, 0:4, :], xT_d.ap()[1][:, 0:4, :]),
                nc.gpsimd.dma_start(xT_sb[:, 1, 4:8, :], xT_d.ap()[1][:, 4:8, :]),
            ]

            # ---- PE warmup on scratch while the first DMAs are in flight.
            # Keeps the HAM clock-gate at 2.4 GHz by the time real work lands.
            for i in range(N_WARM_BIG):
                pw = psum_h.tile([P, 512], f32, tag="ph", name="pwarm")
                nc.tensor.matmul(
                    pw[:], scratch[:, :P], scratch[:], start=True, stop=True
                )
            for i in range(N_WARM_SMALL):
                pw = psum_h.tile([P, 512], f32, tag="ph", name="pwarm")
                nc.tensor.matmul(
                    pw[:, :P], scratch[:, :P], scratch[:, :P], start=True, stop=True
                )

            h_sb = hpool.tile([P, FT, T], f16)

            # ---- phase 1, chunk-major: h^T = gelu(w1^T x^T + b1).
            # All w1 tiles stay resident, so chunk 0 only needs x chunk 0 and
            # w1[ft] just-in-time; x chunk 1 has ~110us of slack.
            gelu_insts = {}
            for c in range(NCHUNK):
                cs = slice(c * 512, (c + 1) * 512)
                for ft in range(FT):
                    ph = psum_h.tile([P, 512], f32, tag="ph", name="ph")
                    for do in range(DO):
                        nc.tensor.matmul(
                            ph[:],
                            w1_tiles[ft][:, do, :],
                            xT_sb[:, c, do, :],
                            start=(do == 0),
                            stop=(do == DO - 1),
                        )
                    gelu_insts[(ft, c)] = nc.scalar.activation(
                        h_sb[:, ft, cs], ph[:], gelu, bias=b1_sb[:, ft : ft + 1]
                    )

            # Pace the non-critical input DMAs behind phase-1 progress: w1[ft]
            # issues once f-tile ft-5 of chunk 0 is done (5 tiles of
            # lookahead), x chunk 1 once the head has cleared. Without this
            # their packets queue ahead of the just-in-time critical tiles.
            for ft in range(5, FT):
                add_dep_helper(
                    w1_dmas[ft].ins,
                    gelu_insts[(ft - 5, 0)].ins,
                    sync=True,
                    reason="pace w1 stream to phase-1 consumption",
                )
            for dma in xc1_dmas:
                add_dep_helper(
                    dma.ins,
                    gelu_insts[(16, 0)].ins,
                    sync=True,
                    reason="keep x chunk 1 out of the head window",
                )

            # ---- phase 2: y^T[dmo] = w2[:,dmo]^T h^T + b2[dmo]
            FQ = FT // 4
            for dmo in range(DMO):
                w2_q = []
                for qq in range(4):
                    w2_sb = wpool.tile([P, FQ, P], f16, tag="w2", bufs=8, name="w2_sb")
                    dma = nc.gpsimd.dma_start(
                        w2_sb[:],
                        w2_d.ap()[dmo, qq // 2, :, (qq % 2) * FQ : (qq % 2 + 1) * FQ],
                    )
                    if dmo == 0 and qq == 0:
                        # keep the w2 stream out of the head's w1/x DMA window
                        add_dep_helper(
                            dma.ins,
                            gelu_insts[(28, 0)].ins,
                            sync=True,
                            reason="delay w2 prefetch past the kernel head",
                        )
                    w2_q.append(w2_sb)

                if dmo < DMO - 1:
                    py = psum_y.tile([P, T], f32, tag="py", name="py")
                    for fo in range(FT):
                        wt = w2_q[fo // FQ][:, fo % FQ, :]
                        for c in range(NCHUNK):
                            cs = slice(c * 512, (c + 1) * 512)
                            nc.tensor.matmul(
                                py[:, cs],
                                wt,
                                h_sb[:, fo, cs],
                                start=(fo == 0),
                                stop=(fo == FT - 1),
                            )
                    # bias-add + store in 256 chunks so the DMA overlaps the add
                    for cq in range(4):
                        cs = slice(cq * 256, (cq + 1) * 256)
                        y_sb = ypool.tile([P, 256], f32, tag="y", bufs=4, name="y_sb")
                        nc.vector.tensor_scalar_add(
                            y_sb[:], py[:, cs], b2_sb[:, dmo : dmo + 1]
                        )
                        nc.sync.dma_start(yT_d.ap()[dmo, :, cs], y_sb[:])
                else:
                    # last dm-tile: chunk-major on 1-bank psum tiles (from the
                    # phase-1 pool, long dead) so chunk 0's epilogue overlaps
                    # chunk 1's matmuls. The final adds run on vector+scalar
                    # in parallel; all stores issue from sync (gpsimd wakes up
                    # too slowly for the tail-critical store).
                    def last_mm_chunk(c):
                        py_c = psum_h.tile([P, 512], f32, tag="ph", name="py_c")
                        mms = []
                        for fo in range(FT):
                            wt = w2_q[fo // FQ][:, fo % FQ, :]
                            mms.append(
                                nc.tensor.matmul(
                                    py_c[:],
                                    wt,
                                    h_sb[:, fo, c * 512 : (c + 1) * 512],
                                    start=(fo == 0),
                                    stop=(fo == FT - 1),
                                )
                            )
                        return py_c, mms

                    def last_flush(py_c, c):
                        for half in range(2):
                            hs = slice(half * 256, (half + 1) * 256)
                            ds = slice(c * 512 + half * 256, c * 512 + half * 256 + 256)
                            y_sb = ypool.tile(
                                [P, 256], f32, tag="y", bufs=4, name="y_sb"
                            )
                            if half == 0:
                                nc.vector.tensor_scalar_add(
                                    y_sb[:], py_c[:, hs], b2_sb[:, dmo : dmo + 1]
                                )
                                nc.sync.dma_start(yT_d.ap()[dmo, :, ds], y_sb[:])
                            else:
                                nc.scalar.activation(
                                    y_sb[:], py_c[:, hs], ident,
                                    bias=b2_sb[:, dmo : dmo + 1],
                                )
                                nc.scalar.dma_start(yT_d.ap()[dmo, :, ds], y_sb[:])

                    py_c0, _ = last_mm_chunk(0)
                    last_flush(py_c0, 0)
                    py_c1, c1_mms = last_mm_chunk(1)
                    # ring-warm trickles: tiny loads issued mid-way through
                    # chunk 1's matmuls keep the DMA rings awake so the
                    # tail-critical final stores skip the wake-up latency.
                    warm_sb = cpool.tile([P, 16], f16, name="warm_sb")
                    for k, gate in ((0, 12), (1, 22)):
                        tdma = nc.sync.dma_start(
                            warm_sb[:, 8 * k : 8 * k + 8],
                            xT_d.ap()[0][:, 0, 8 * k : 8 * k + 8],
                        )
                        add_dep_helper(
                            tdma.ins,
                            c1_mms[gate].ins,
                            sync=True,
                            reason="ring-warm trickle before final store",
                        )
                    last_flush(py_c1, 1)

    nc.compile()
    return nc


def _get_nc():
    global _cached
    if _cached is None:
        _cached = _build()
    return _cached


def make_in_maps(x, w1, b1, w2, b2):
    x = np.asarray(x, dtype=np.float32)
    w1 = np.asarray(w1, dtype=np.float32)
    b1 = np.asarray(b1, dtype=np.float32)
    w2 = np.asarray(w2, dtype=np.float32)
    b2 = np.asarray(b2, dtype=np.float32)

    tokens = x.reshape(E, T, D)
    in_maps = []
    for e in range(E):
        xT = np.ascontiguousarray(
            tokens[e].reshape(NCHUNK, 512, DO, P).transpose(0, 3, 2, 1)
        ).astype(np.float16)  # [c, p, do, t']
        w1r = np.ascontiguousarray(
            w1[e].reshape(DO, P, FT, P).transpose(2, 1, 0, 3)
        ).astype(np.float16)  # [ft, p, do, j]
        bc = np.ascontiguousarray(
            np.concatenate([b1[e].reshape(FT, P).T, b2[e].reshape(DMO, P).T], axis=1)
        )  # [p, ft..dmo]
        w2r = np.ascontiguousarray(
            w2[e].reshape(2, FT2, P, DMO, P).transpose(3, 0, 2, 1, 4)
        ).astype(np.float16)  # [dmo, half, p, fo, j]
        in_maps.append({"xT": xT, "w1r": w1r, "bc": bc, "w2r": w2r})
    return in_maps


def gather_out(results):
    out = np.empty((E, T, D), dtype=np.float32)
    for e in range(E):
        yT = results[e]["yT"]  # [dmo, p, t]
        out[e] = yT.transpose(2, 0, 1).reshape(T, D)
    return out.reshape(B, S, D)


def kernel(x, w1, b1, w2, b2):
    from concourse.bass_utils import run_bass_kernel_spmd

    nc = _get_nc()
    in_maps = make_in_maps(x, w1, b1, w2, b2)
    res = run_bass_kernel_spmd(nc, in_maps, core_ids=list(range(E)))
    return gather_out(res.results)


# revision 17
# speedup vs baseline: 1.0661x; 1.0412x over previous
"""Trainium2 Bass kernel for an 8-expert MoE FFN layer (nn_MoELayer).

Reference computation (per expert e over its contiguous 1024-token chunk):
    h = gelu(x_e @ w1[e] + b1[e]);  y_e = h @ w2[e] + b2[e]

Sharding: expert parallelism — core e holds expert e's weights and its token
chunk (the gate yields equal contiguous chunks, so no all-to-all is needed).
Each core runs the same SPMD program on its own data.

Per-core kernel (T=1024 tokens, D=1024, F=4096), all matmuls in fp16 with
fp32 PSUM accumulation (~216 ns per 512-wide matmul incl. hidden weight load —
the PE's floor). 1024 matmuls -> ~220.7 us of PE stream; everything else is
head/tail overlap engineering:
  - warmup matmuls on (uninitialized) scratch start right after the engine
    preamble, so HAM un-throttles the PE clock before real data lands
  - critical input DMAs (w1[0..2], x chunk-0 quarters) issue in parallel from
    four different engine queues to cut descriptor-issue serialization
  - phase 1 runs chunk-major (all 32 f-tiles on chunk 0, then chunk 1) with
    all of w1 SBUF-resident, so the head only waits for w1[0] + x chunk-0
    quarters (~0.5 MB) instead of both chunks (~2.25 MB)
  - phase 2: per dm-tile 2-bank PSUM accumulate; the last dm-tile runs
    chunk-major on 1-bank psum tiles so its first chunk's epilogue overlaps
    the second chunk's matmuls, and the final flush is split across the
    vector+scalar engines with DMA issue on sync+gpsimd in parallel.
"""

import os

import numpy as np

# The kernel executes through the axon PJRT backend; a CPU pin (e.g. set for
# a jax reference run) would break NEFF dispatch in this process.
if os.environ.get("JAX_PLATFORMS") == "cpu":
    del os.environ["JAX_PLATFORMS"]

E = 8          # experts == cores
B, S = 2, 4096
D = 1024       # d_model
F = 4096       # d_ff
T = (B * S) // E  # tokens per expert chunk = 1024
P = 128
DO = D // P    # 8  k-tiles of d_model
FT = F // P    # 32 f-tiles of d_ff
DMO = D // P   # 8  output dm-tiles
FT2 = FT // 2  # half-slab of w2 f-tiles
NCHUNK = T // 512  # 2 moving-operand chunks (PSUM bank caps matmul N at 512)
N_WARM_BIG = 8     # N=512 warmups: ~3.4us cold -> trips the HAM un-throttle
N_WARM_SMALL = 6   # N=128 warmups: fine-grained handoff to the first real mm
# fp8 DoubleRow runs the PE at ~1.5-1.8x fp16 rate but quantizes both
# operands to e4m3 (~4% relative error on the affected h columns). Applying
# it to K8 of the 32 phase-1 f-tiles costs ~4.04%*sqrt(K8/32) of final
# rel_l2 error (measured: 6/32 -> 1.75e-2 vs the 2e-2 budget) and saves
# ~0.6-0.8us of PE time per converted tile.
K8 = 6             # number of fp8-DoubleRow phase-1 f-tiles (error knob)
FT16 = FT - K8     # f-tiles that stay fp16
DP = DO // 2       # d-tile pairs (DoubleRow contracts 256 rows per matmul)

_cached = None


def _build():
    import concourse.mybir as mybir
    import concourse.tile as tile
    from concourse import bacc
    from concourse.tile_rust import add_dep_helper

    f32 = mybir.dt.float32
    f16 = mybir.dt.float16
    f8 = mybir.dt.float8e4
    dblrow = mybir.MatmulPerfMode.DoubleRow

    nc = bacc.Bacc("TRN2", target_bir_lowering=False, debug=False, num_devices=E)

    xT_d = nc.dram_tensor("xT", [NCHUNK, P, DO, 512], f16, kind="ExternalInput")
    w1_d = nc.dram_tensor("w1r", [FT16, P, DO, P], f16, kind="ExternalInput")
    x8_d = nc.dram_tensor("x8", [NCHUNK, P, DP, 2, 512], f8, kind="ExternalInput")
    w18_d = nc.dram_tensor("w18", [K8, P, DP, 2, P], f8, kind="ExternalInput")
    bc_d = nc.dram_tensor("bc", [P, FT + DMO], f32, kind="ExternalInput")
    w2_d = nc.dram_tensor("w2r", [DMO, 2, P, FT2, P], f16, kind="ExternalInput")
    yT_d = nc.dram_tensor("yT", [DMO, P, T], f32, kind="ExternalOutput")

    gelu = mybir.ActivationFunctionType.Gelu_apprx_tanh
    ident = mybir.ActivationFunctionType.Identity

    with tile.TileContext(nc) as tc:
        with (
            tc.tile_pool(name="xpool", bufs=1) as xpool,
            tc.tile_pool(name="hpool", bufs=1) as hpool,
            tc.tile_pool(name="wpool", bufs=2) as wpool,
            tc.tile_pool(name="cpool", bufs=1) as cpool,
            tc.tile_pool(name="ypool", bufs=2) as ypool,
            tc.tile_pool(name="psum_h", bufs=4, space="PSUM") as psum_h,
            tc.tile_pool(name="psum_y", bufs=2, space="PSUM") as psum_y,
        ):
            # Scratch for PE warmup, zeroed on the vector engine (idle at the
            # head, and not a DMA-capable engine) so the warmup matmuls can
            # start right after the preamble without gating any input DMA.
            scratch = cpool.tile([P, 512], f16)
            nc.vector.memset(scratch[:], 0.0)

            # ---- critical input DMAs, issued in parallel across engines.
            # gpsimd: w1[0..2] + x chunk 1; vector/scalar: x chunk-0 quarters.
            # sync: the long w1 stream. One engine issuing everything would
            # serialize ~600ns per descriptor right on the critical path.
            w1_tiles = []
            for ft in range(FT16):
                w1_tiles.append(
                    wpool.tile([P, DO, P], f16, tag="w1", bufs=FT16, name="w1_sb")
                )
            w18_tiles = []
            for k in range(K8):
                w18_tiles.append(
                    wpool.tile([P, DP, 2, P], f8, tag="w18", bufs=K8, name="w18_sb")
                )
            xT_sb = xpool.tile([P, NCHUNK, DO, 512], f16)
            x8_sb = xpool.tile([P, NCHUNK, DP, 2, 512], f8)

            # Strict need-order across the two fastest-issuing engines: the
            # first matmul group consumes (w1[0] do-half 0, x q0, q1, w1[0]
            # do-half 1, q2, q3), then w1[1], w1[2], ... Everything beyond a
            # few tiles of lookahead is gated behind phase-1 progress (below)
            # so its packets can't crowd the critical head window.
            nc.scalar.dma_start(w1_tiles[0][:, 0:4, :], w1_d.ap()[0][:, 0:4, :])
            nc.sync.dma_start(xT_sb[:, 0, 0:2, :], xT_d.ap()[0][:, 0:2, :])
            nc.scalar.dma_start(xT_sb[:, 0, 2:4, :], xT_d.ap()[0][:, 2:4, :])
            nc.sync.dma_start(w1_tiles[0][:, 4:8, :], w1_d.ap()[0][:, 4:8, :])
            nc.scalar.dma_start(xT_sb[:, 0, 4:6, :], xT_d.ap()[0][:, 4:6, :])
            nc.sync.dma_start(xT_sb[:, 0, 6:8, :], xT_d.ap()[0][:, 6:8, :])
            nc.scalar.dma_start(w1_tiles[1][:], w1_d.ap()[1])
            nc.sync.dma_start(w1_tiles[2][:], w1_d.ap()[2])
            bc_sb = cpool.tile([P, FT + DMO], f32)
            nc.scalar.dma_start(bc_sb[:], bc_d.ap())
            w1_dmas = {}
            for ft in range(3, FT):
                if ft < FT16:
                    w1_dmas[ft] = nc.sync.dma_start(w1_tiles[ft][:], w1_d.ap()[ft])
                else:
                    w1_dmas[ft] = nc.sync.dma_start(
                        w18_tiles[ft - FT16][:], w18_d.ap()[ft - FT16]
                    )
            b1_sb = bc_sb[:, :FT]
            b2_sb = bc_sb[:, FT:]
            # x chunk 1 and the fp8 copy of x are only needed later in
            # phase 1; gpsimd issues them once the head has cleared (gated
            # below).
            x8c0_dma = nc.gpsimd.dma_start(x8_sb[:, 0], x8_d.ap()[0])
            xc1_dmas = [
                nc.gpsimd.dma_start(xT_sb[:, 1, 0:4, :], xT_d.ap()[1][:, 0:4, :]),
                nc.gpsimd.dma_start(xT_sb[:, 1, 4:8, :], xT_d.ap()[1][:, 4:8, :]),
                nc.gpsimd.dma_start(x8_sb[:, 1], x8_d.ap()[1]),
            ]

            # ---- PE warmup on scratch while the first DMAs are in flight.
            # Keeps the HAM clock-gate at 2.4 GHz by the time real work lands.
            for i in range(N_WARM_BIG):
                pw = psum_h.tile([P, 512], f32, tag="ph", name="pwarm")
                nc.tensor.matmul(
                    pw[:], scratch[:, :P], scratch[:], start=True, stop=True
                )
            for i in range(N_WARM_SMALL):
                pw = psum_h.tile([P, 512], f32, tag="ph", name="pwarm")
                nc.tensor.matmul(
                    pw[:, :P], scratch[:, :P], scratch[:, :P], start=True, stop=True
                )

            h_sb = hpool.tile([P, FT, T], f16)

            # ---- phase 1, chunk-major: h^T = gelu(w1^T x^T + b1).
            # All w1 tiles stay resident, so chunk 0 only needs x chunk 0 and
            # w1[ft] just-in-time; x chunk 1 has ~110us of slack.
            gelu_insts = {}
            for c in range(NCHUNK):
                cs = slice(c * 512, (c + 1) * 512)
                for ft in range(FT):
                    ph = psum_h.tile([P, 512], f32, tag="ph", name="ph")
                    if ft < FT16:
                        for do in range(DO):
                            nc.tensor.matmul(
                                ph[:],
                                w1_tiles[ft][:, do, :],
                                xT_sb[:, c, do, :],
                                start=(do == 0),
                                stop=(do == DO - 1),
                            )
                    else:
                        # fp8 DoubleRow: each matmul contracts a 256-row
                        # d-tile pair (2 fp8 weights per PE cell)
                        for dp in range(DP):
                            nc.tensor.matmul(
                                ph[:],
                                w18_tiles[ft - FT16][:, dp, :, :],
                                x8_sb[:, c, dp, :, :],
                                start=(dp == 0),
                                stop=(dp == DP - 1),
                                perf_mode=dblrow,
                            )
                    gelu_insts[(ft, c)] = nc.scalar.activation(
                        h_sb[:, ft, cs], ph[:], gelu, bias=b1_sb[:, ft : ft + 1]
                    )

            # Pace the non-critical input DMAs behind phase-1 progress: w1[ft]
            # issues once f-tile ft-5 of chunk 0 is done (5 tiles of
            # lookahead), x chunk 1 once the head has cleared. Without this
            # their packets queue ahead of the just-in-time critical tiles.
            for ft in range(5, FT):
                add_dep_helper(
                    w1_dmas[ft].ins,
                    gelu_insts[(ft - 5, 0)].ins,
                    sync=True,
                    reason="pace w1 stream to phase-1 consumption",
                )
            add_dep_helper(
                x8c0_dma.ins,
                gelu_insts[(10, 0)].ins,
                sync=True,
                reason="keep fp8 x chunk 0 out of the head window",
            )
            for dma in xc1_dmas:
                add_dep_helper(
                    dma.ins,
                    gelu_insts[(16, 0)].ins,
                    sync=True,
                    reason="keep x chunk 1 out of the head window",
                )

            # ---- phase 2: y^T[dmo] = w2[:,dmo]^T h^T + b2[dmo]
            FQ = FT // 4
            for dmo in range(DMO):
                w2_q = []
                for qq in range(4):
                    w2_sb = wpool.tile([P, FQ, P], f16, tag="w2", bufs=8, name="w2_sb")
                    dma = nc.gpsimd.dma_start(
                        w2_sb[:],
                        w2_d.ap()[dmo, qq // 2, :, (qq % 2) * FQ : (qq % 2 + 1) * FQ],
                    )
                    if dmo == 0 and qq == 0:
                        # keep the w2 stream out of the head's w1/x DMA window
                        add_dep_helper(
                            dma.ins,
                            gelu_insts[(28, 0)].ins,
                            sync=True,
                            reason="delay w2 prefetch past the kernel head",
                        )
                    w2_q.append(w2_sb)

                if dmo < DMO - 1:
                    py = psum_y.tile([P, T], f32, tag="py", name="py")
                    for fo in range(FT):
                        wt = w2_q[fo // FQ][:, fo % FQ, :]
                        for c in range(NCHUNK):
                            cs = slice(c * 512, (c + 1) * 512)
                            nc.tensor.matmul(
                                py[:, cs],
                                wt,
                                h_sb[:, fo, cs],
                                start=(fo == 0),
                                stop=(fo == FT - 1),
                            )
                    # bias-add + store in 256 chunks so the DMA overlaps the add
                    for cq in range(4):
                        cs = slice(cq * 256, (cq + 1) * 256)
                        y_sb = ypool.tile([P, 256], f32, tag="y", bufs=4, name="y_sb")
                        nc.vector.tensor_scalar_add(
                            y_sb[:], py[:, cs], b2_sb[:, dmo : dmo + 1]
                        )
                        nc.sync.dma_start(yT_d.ap()[dmo, :, cs], y_sb[:])
                else:
                    # last dm-tile: chunk-major on 1-bank psum tiles (from the
                    # phase-1 pool, long dead) so chunk 0's epilogue overlaps
                    # chunk 1's matmuls. The final adds run on vector+scalar
                    # in parallel; all stores issue from sync (gpsimd wakes up
                    # too slowly for the tail-critical store).
                    def last_mm_chunk(c):
                        py_c = psum_h.tile([P, 512], f32, tag="ph", name="py_c")
                        mms = []
                        for fo in range(FT):
                            wt = w2_q[fo // FQ][:, fo % FQ, :]
                            mms.append(
                                nc.tensor.matmul(
                                    py_c[:],
                                    wt,
                                    h_sb[:, fo, c * 512 : (c + 1) * 512],
                                    start=(fo == 0),
                                    stop=(fo == FT - 1),
                                )
                            )
                        return py_c, mms

                    def last_flush(py_c, c):
                        for half in range(2):
                            hs = slice(half * 256, (half + 1) * 256)
                            ds = slice(c * 512 + half * 256, c * 512 + half * 256 + 256)
                            y_sb = ypool.tile(
                                [P, 256], f32, tag="y", bufs=4, name="y_sb"
                            )
                            if half == 0:
                                nc.vector.tensor_scalar_add(
                                    y_sb[:], py_c[:, hs], b2_sb[:, dmo : dmo + 1]
                                )
                                nc.sync.dma_start(yT_d.ap()[dmo, :, ds], y_sb[:])
                            else:
                                nc.scalar.activation(
                                    y_sb[:], py_c[:, hs], ident,
                                    bias=b2_sb[:, dmo : dmo + 1],
                                )
                                nc.scalar.dma_start(yT_d.ap()[dmo, :, ds], y_sb[:])

                    py_c0, _ = last_mm_chunk(0)
                    last_flush(py_c0, 0)
                    py_c1, c1_mms = last_mm_chunk(1)
                    # ring-warm trickles: tiny loads issued mid-way through
                    # chunk 1's matmuls keep the DMA rings awake so the
                    # tail-critical final stores skip the wake-up latency.
                    warm_sb = cpool.tile([P, 16], f16, name="warm_sb")
                    for k, gate in ((0, 12), (1, 22)):
                        tdma = nc.sync.dma_start(
                            warm_sb[:, 8 * k : 8 * k + 8],
                            xT_d.ap()[0][:, 0, 8 * k : 8 * k + 8],
                        )
                        add_dep_helper(
                            tdma.ins,
                            c1_mms[gate].ins,
                            sync=True,
                            reason="ring-warm trickle before final store",
                        )
                    last_flush(py_c1, 1)

    nc.compile()
    return nc


def _get_nc():
    global _cached
    if _cached is None:
        _cached = _build()
    return _cached


def make_in_maps(x, w1, b1, w2, b2):
    x = np.asarray(x, dtype=np.float32)
    w1 = np.asarray(w1, dtype=np.float32)
    b1 = np.asarray(b1, dtype=np.float32)
    w2 = np.asarray(w2, dtype=np.float32)
    b2 = np.asarray(b2, dtype=np.float32)

    import ml_dtypes

    f8 = ml_dtypes.float8_e4m3

    tokens = x.reshape(E, T, D)
    in_maps = []
    for e in range(E):
        xT = np.ascontiguousarray(
            tokens[e].reshape(NCHUNK, 512, DO, P).transpose(0, 3, 2, 1)
        ).astype(np.float16)  # [c, p, do, t']
        x8 = np.ascontiguousarray(
            tokens[e].reshape(NCHUNK, 512, DP, 2, P).transpose(0, 4, 2, 3, 1)
        ).astype(f8)  # [c, p, dp, ko, t']
        w1r = np.ascontiguousarray(
            w1[e].reshape(DO, P, FT, P).transpose(2, 1, 0, 3)[:FT16]
        ).astype(np.float16)  # [ft, p, do, j]
        w18 = np.ascontiguousarray(
            w1[e].reshape(DP, 2, P, FT, P).transpose(3, 2, 0, 1, 4)[FT16:]
        ).astype(f8)  # [ft8, p, dp, ko, j]
        bc = np.ascontiguousarray(
            np.concatenate([b1[e].reshape(FT, P).T, b2[e].reshape(DMO, P).T], axis=1)
        )  # [p, ft..dmo]
        w2r = np.ascontiguousarray(
            w2[e].reshape(2, FT2, P, DMO, P).transpose(3, 0, 2, 1, 4)
        ).astype(np.float16)  # [dmo, half, p, fo, j]
        in_maps.append(
            {"xT": xT, "x8": x8, "w1r": w1r, "w18": w18, "bc": bc, "w2r": w2r}
        )
    return in_maps


def gather_out(results):
    out = np.empty((E, T, D), dtype=np.float32)
    for e in range(E):
        yT = results[e]["yT"]  # [dmo, p, t]
        out[e] = yT.transpose(2, 0, 1).reshape(T, D)
    return out.reshape(B, S, D)


def kernel(x, w1, b1, w2, b2):
    from concourse.bass_utils import run_bass_kernel_spmd

    nc = _get_nc()
    in_maps = make_in_maps(x, w1, b1, w2, b2)
    res = run_bass_kernel_spmd(nc, in_maps, core_ids=list(range(E)))
    return gather_out(res.results)
